# revision 41
# baseline (speedup 1.0000x reference)
"""Trainium2 Bass kernel for nn_Block_50706383897045 (dense transformer block).

Data-parallel over batch: B=8 == n_cores, one batch element per core, no
collectives. Per core the block runs on a [T=1024, C=768] slice.

v1 restructure (vs the staged baseline): the kernel is emission-interleaved
as a chunk-level software pipeline so the PE never starves (HAM stays at
2.4 GHz):
  A  token-major LN1 on DVE (bn_stats/bn_aggr, no PE stats, no row recips)
     + PE transposes of x and h to feature-major, zipped with chunk-0
     K/Q/V projections.
  C  attention chunk0 zipped with chunk-1 K/Q/V projections.
  D  attention chunk1 zipped with out_proj/LN2/MLP-c1-pass of chunk0.
  E  gelu batches, MLP c2 passes, chunk-1 MLP, stores.
Other changes: causal column restriction in scores/exp/attn.v (-25%),
softmax denominators via batched reciprocal_approx_fast + PE outer-product
broadcast (no 3.3us single-lane DVE recips, no gpsimd broadcasts), rsqrt
via Log/Exp on ACT (stays in the exp table set -> no ACT table thrash),
gelu deferred out of the exp region (2 table loads total), bf16 for
V/out/MLP weights and moving activations (halves SBUF + DMA; PE column
rate is dtype-independent so accuracy is spent only where it buys space).
Scores path (k,q) stays f32r for exp precision.
"""
import os
import sys

sys.path.insert(0, "/opt/trn_rl_repo")

PHASES = os.environ.get("KPH", "ACDE")

import ml_dtypes
import numpy as np

import concourse.bass as bass
import concourse.bacc as bacc
import concourse.mybir as mybir
import concourse.tile as tile
from concourse import bass_utils
from concourse.masks import make_identity

AF = mybir.ActivationFunctionType
ALU = mybir.AluOpType
f32 = mybir.dt.float32
f32r = mybir.dt.float32r
bf16 = mybir.dt.bfloat16

B, T, C, H, D = 8, 1024, 768, 12, 64
F = C // 128      # 6 feature tiles of the residual stream
NT = T // 128     # 8 token tiles
CH = 512          # token chunk
NCH = 2
M3 = 4 * C        # 3072 MLP hidden
MT = M3 // 128    # 24 MLP hidden tiles
EPS = 1e-5

_NC_CACHE = None


def _interleave(steps, fillers):
    """Emit steps and fillers interleaved so both lists finish together."""
    ns, nf = len(steps), len(fillers)
    fi = 0
    for si, s in enumerate(steps):
        s()
        target = (si + 1) * nf // max(ns, 1)
        while fi < target:
            fillers[fi]()
            fi += 1
    while fi < nf:
        fillers[fi]()
        fi += 1


def _build():
    nc = bacc.Bacc("TRN2", target_bir_lowering=False, debug=False,
                   num_devices=8)
    dd = {
        "x": nc.dram_tensor("x", [T, C], f32, kind="ExternalInput"),
        "w_kq": nc.dram_tensor("w_kq", [C, 2 * C], f32, kind="ExternalInput"),
        "w_v": nc.dram_tensor("w_v", [C, C], f32, kind="ExternalInput"),
        "b_qkv": nc.dram_tensor("b_qkv", [3 * C], f32, kind="ExternalInput"),
        "w_out": nc.dram_tensor("w_out", [C, C], bf16, kind="ExternalInput"),
        "b_out": nc.dram_tensor("b_out", [C], f32, kind="ExternalInput"),
        "w_c1": nc.dram_tensor("w_c1", [C, M3], bf16, kind="ExternalInput"),
        "b_c1": nc.dram_tensor("b_c1", [M3], f32, kind="ExternalInput"),
        "w_c2": nc.dram_tensor("w_c2", [M3, C], bf16, kind="ExternalInput"),
        "b_c2": nc.dram_tensor("b_c2", [C], f32, kind="ExternalInput"),
        "ln_w": nc.dram_tensor("ln_w", [C], f32, kind="ExternalInput"),
        "ln_b": nc.dram_tensor("ln_b", [C], f32, kind="ExternalInput"),
        "y": nc.dram_tensor("y", [T, C], f32, kind="ExternalOutput"),
    }
    with tile.TileContext(nc) as tc:
        _body(nc, tc, dd)
    nc.compile()
    return nc


def _col_rearr(ap, p=128):
    return ap.rearrange("(o p) -> p o", p=p)


def _body(nc, tc, dd):
    x_d, y_d = dd["x"], dd["y"]
    with tc.tile_pool(name="persist", bufs=1) as pp:
        ident = pp.tile([128, 128], f32, name="ident")
        make_identity(nc, ident)
        ones1 = pp.tile([1, 128], f32r, name="ones1")
        nc.vector.memset(ones1.bitcast(f32), 1.0)
        ones_col = pp.tile([128, 1], f32r, name="ones_col")
        nc.vector.memset(ones_col.bitcast(f32), 1.0)

        eps128 = pp.tile([128, 1], f32, name="eps128")
        nc.vector.memset(eps128, EPS)
        eps1 = pp.tile([1, 1], f32, name="eps1")
        nc.vector.memset(eps1, EPS)
        lnw_c = pp.tile([128, F], f32, name="lnw_c")
        nc.sync.dma_start(lnw_c, _col_rearr(dd["ln_w"].ap()))
        lnb_c = pp.tile([128, F], f32, name="lnb_c")
        nc.sync.dma_start(lnb_c, _col_rearr(dd["ln_b"].ap()))
        bkq_c = pp.tile([128, 12], f32, name="bkq_c")
        nc.sync.dma_start(bkq_c, _col_rearr(dd["b_qkv"].ap()[0:2 * C]))
        bout_c = pp.tile([128, F], f32, name="bout_c")
        nc.sync.dma_start(bout_c, _col_rearr(dd["b_out"].ap()))
        bc1_c = pp.tile([128, MT], f32, name="bc1_c")
        nc.sync.dma_start(bc1_c, _col_rearr(dd["b_c1"].ap()))
        bc2_c = pp.tile([128, F], f32, name="bc2_c")
        nc.sync.dma_start(bc2_c, _col_rearr(dd["b_c2"].ap()))
        # V bias broadcast along partitions: [128, C]
        bv_bc = pp.tile([128, C], f32, name="bv_bc")
        bv_src = dd["b_qkv"].ap()[2 * C:3 * C]
        bv_b = bass.AP(tensor=bv_src.tensor, offset=bv_src.offset,
                       ap=[[0, 128]] + [list(p) for p in bv_src.ap])
        nc.gpsimd.dma_start(out=bv_bc, in_=bv_b)
        # causal 0/1 masks (bf16) for diagonal offsets 0,-128,-256,-384
        masks = pp.tile([128, 4, CH], bf16, name="masks")

        _main(nc, tc, dd, ident, ones1, ones_col, eps128,
              eps1, lnw_c, lnb_c, bkq_c, bout_c, bc1_c, bc2_c, bv_bc, masks)


def _main(nc, tc, dd, ident, ones1, ones_col, eps128, eps1,
          lnw_c, lnb_c, bkq_c, bout_c, bc1_c, bc2_c, bv_bc, masks):
    x_d, y_d = dd["x"], dd["y"]
    with (
        tc.tile_pool(name="resid", bufs=1) as rp,
        tc.tile_pool(name="attst", bufs=1) as asp,
        tc.tile_pool(name="rot", bufs=1) as rot,
        tc.tile_pool(name="ps_acc", bufs=2, space="PSUM") as ps_acc,
        tc.tile_pool(name="ps_s", bufs=2, space="PSUM") as ps_s,
        tc.tile_pool(name="ps_y", bufs=2, space="PSUM") as ps_y,
        tc.tile_pool(name="ps_aux", bufs=2, space="PSUM") as ps_aux,
    ):
        x_fm = [rp.tile([128, F, CH], f32, tag=f"xfm{c}", name=f"x_fm{c}")
                for c in range(NCH)]
        x2_fm = rp.tile([128, F, T], f32r, name="x2_fm")
        kf = asp.tile([128, F, T], f32r, name="kf")
        v1 = asp.tile([128, NT, H * 65], bf16, name="v1")
        att_o = [asp.tile([128, F, CH], bf16, tag=f"ao{c}", name=f"ao{c}")
                 for c in range(NCH)]
        nc.vector.memset(
            v1.rearrange("p t (h m) -> p t h m", m=65)[:, :, :, 64:65], 1.0)

        st = {"qf": {}, "pend": {}, "ys": {}, "trslot": [0]}

        def tr_ps_tile(shape=(128, 128)):
            # rotate transpose/aux PSUM slots across the s/y/aux pools
            i = st["trslot"][0]
            st["trslot"][0] = (i + 1) % 3
            pool = (ps_s, ps_y, ps_aux)[i]
            tag = ("s", "y", "aux")[i]
            return pool.tile(list(shape), f32, tag=tag, name="tr")

        # ---------------- phase A + B(c0) ----------------
        with tc.tile_pool(name="fw", bufs=1) as fw:
            wv_t = []
            for kt in range(F):
                wt = fw.tile([128, C], f32r, tag=f"wv{kt}", name=f"wv{kt}")
                nc.sync.dma_start(
                    wt, dd["w_v"].ap().bitcast(f32r)
                    [kt * 128:(kt + 1) * 128, :])
                wv_t.append(wt)
            h_fm = fw.tile([128, F, T], f32r, name="h_fm")

            def v_unit(c, t, half):
                def emit():
                    ps = ps_acc.tile([128, 384], f32, tag="acc", name="ps_v")
                    c0 = half * 384
                    for kt in range(F):
                        nc.tensor.matmul(
                            ps, h_fm[:, kt, t * 128:(t + 1) * 128],
                            wv_t[kt][:, c0:c0 + 384],
                            start=(kt == 0), stop=(kt == F - 1))
                    dst = (v1[:, t, :].rearrange("p (h m) -> p h m", m=65)
                           [:, half * 6:(half + 1) * 6, 0:64])
                    src = ps.rearrange("p (h m) -> p h m", m=64)
                    bias = (bv_bc[:, c0:c0 + 384]
                            .rearrange("p (h m) -> p h m", m=64))
                    nc.vector.tensor_add(dst, src, bias)
                return emit

            def kq_unit(c, f, which):
                def emit():
                    sl = slice(c * CH, (c + 1) * CH)
                    col0 = which * C + f * 128
                    wt = fw.tile([128, F, 128], f32r, tag="wkqt", bufs=2,
                                 name="wkqt")
                    for kt in range(F):
                        nc.sync.dma_start(
                            wt[:, kt, :],
                            dd["w_kq"].ap().bitcast(f32r)
                            [kt * 128:(kt + 1) * 128, col0:col0 + 128])
                    ps = ps_acc.tile([128, CH], f32, tag="acc", name="ps_kq")
                    for kt in range(F):
                        nc.tensor.matmul(
                            ps, wt[:, kt, :], h_fm[:, kt, sl],
                            start=(kt == 0), stop=(kt == F - 1))
                    if which == 0:
                        nc.vector.tensor_scalar_add(
                            kf[:, f, sl], ps, bkq_c[:, f:f + 1])
                    else:
                        qt = rot.tile([128, CH], f32r, tag=f"qf{c}",
                                      bufs=(5 if c == 0 else 6),
                                      name=f"qf{f}_{c}")
                        nc.vector.tensor_scalar_add(
                            qt, ps, bkq_c[:, F + f:F + f + 1])
                        st["qf"][(f, c)] = qt
                return emit

            def b_units(c):
                us = []
                for t in range(4 * c, 4 * (c + 1)):
                    for half in range(2):
                        us.append(v_unit(c, t, half))
                for f in range(F):
                    us.append(kq_unit(c, f, 0))
                    us.append(kq_unit(c, f, 1))
                return us

            with tc.tile_pool(name="ft", bufs=1) as ft:
                # build bf16 masks via a small f32 temp
                for mi in range(4):
                    mtmp = ft.tile([128, CH], f32, tag="htm", bufs=2,
                                   name="mtmp")
                    nc.vector.memset(mtmp, 1.0)
                    nc.gpsimd.affine_select(
                        out=mtmp, in_=mtmp,
                        compare_op=ALU.is_ge, fill=0.0,
                        base=-mi * 128, pattern=[[1, CH]],
                        channel_multiplier=-1)
                    nc.vector.tensor_copy(masks[:, mi, :], mtmp)

                def a_unit(t):
                    def emit():
                        x_tm = ft.tile([128, C], f32, tag="xtm", bufs=2,
                                       name=f"x_tm{t}")
                        nc.sync.dma_start(
                            x_tm, x_d.ap()[t * 128:(t + 1) * 128, :])
                        bst = ft.tile([128, 3, 6], f32, tag="bnst", bufs=2,
                                      name="bst")
                        for g in range(3):
                            nc.vector.bn_stats(
                                bst[:, g, :], x_tm[:, g * 256:(g + 1) * 256])
                        mv = ft.tile([128, 2], f32, tag="mv", bufs=2,
                                     name="mv")
                        nc.vector.bn_aggr(mv, bst)
                        lv = ft.tile([128, 1], f32, tag="lv", bufs=2,
                                     name="lv")
                        nc.scalar.activation(lv, mv[:, 1:2], AF.Ln,
                                             bias=eps128)
                        rstd = ft.tile([128, 1], f32, tag="rstd", bufs=2,
                                       name="rstd")
                        nc.scalar.activation(rstd, lv, AF.Exp, scale=-0.5)
                        negmu = ft.tile([128, 1], f32, tag="negmu", bufs=2,
                                        name="negmu")
                        nc.vector.tensor_scalar_mul(negmu, mv[:, 0:1], -1.0)
                        h_tm = ft.tile([128, C], f32, tag="htm", bufs=2,
                                       name="h_tm")
                        nc.vector.tensor_scalar(
                            out=h_tm, in0=x_tm, scalar1=negmu, scalar2=rstd,
                            op0=ALU.add, op1=ALU.mult)
                        c, tj = t // 4, t % 4
                        for f in range(F):
                            fs = slice(f * 128, (f + 1) * 128)
                            ps1 = tr_ps_tile()
                            nc.tensor.transpose(ps1, x_tm[:, fs], ident)
                            nc.scalar.activation(
                                x_fm[c][:, f, tj * 128:(tj + 1) * 128],
                                ps1, AF.Copy)
                            ps2 = tr_ps_tile()
                            nc.tensor.transpose(ps2, h_tm[:, fs], ident)
                            nc.vector.tensor_scalar(
                                out=h_fm[:, f, t * 128:(t + 1) * 128],
                                in0=ps2, scalar1=lnw_c[:, f:f + 1],
                                scalar2=lnb_c[:, f:f + 1],
                                op0=ALU.mult, op1=ALU.add)
                    return emit

                with nc.named_scope("phA"):
                    for t in range(4):
                        a_unit(t)()
                    _interleave([a_unit(t) for t in range(4, 8)],
                                b_units(0))

            # ---------------- attention machinery ----------------
            # NOTE: heads run sequentially; alternating base-0/base-64
            # scores matmuls back-to-back silently corrupts the PE output
            # on this hardware (row-group concurrency issue).
            def _attnv1(f, c, hl, kt, e, sub):
                ktmax = 4 * (c + 1)
                h = 2 * f + hl
                nc.tensor.matmul(
                    st["ys"][(f, c, hl)][:, sub],
                    v1[:, kt, h * 65:(h + 1) * 65], e[:, sub],
                    start=(kt == 0), stop=(kt == ktmax - 1))

            def attn_step1(f, c, kt, hl):
                def emit():
                    off = c * CH - kt * 128
                    lo = max(0, -off)
                    sub = slice(lo, CH)
                    qt = st["qf"][(f, c)]
                    if kt == 0:
                        st["ys"][(f, c, hl)] = ps_y.tile(
                            [65, CH], f32, tag="y", name=f"y{f}_{c}_{hl}")
                    ps = ps_s.tile([128, CH], f32, tag="s", name="s")
                    nc.tensor.matmul(
                        ps[:, sub],
                        kf[hl * 64:(hl + 1) * 64, f,
                           kt * 128:(kt + 1) * 128],
                        qt[hl * 64:(hl + 1) * 64, sub],
                        start=True, stop=True)
                    e = rot.tile([128, CH], bf16, tag="e", bufs=3,
                                 name="expt")
                    nc.scalar.activation(e[:, sub], ps[:, sub], AF.Exp)
                    if off < 128:
                        mi = (-off) // 128
                        nc.vector.tensor_mul(
                            e[:, sub], e[:, sub], masks[:, mi, sub])
                    prev = st["pend"].pop((f, c, hl), None)
                    if prev is not None:
                        _attnv1(f, c, hl, *prev)
                    st["pend"][(f, c, hl)] = (kt, e, sub)
                return emit

            def attn_flush1(f, c, hl):
                def emit():
                    prev = st["pend"].pop((f, c, hl))
                    _attnv1(f, c, hl, *prev)
                    ys = st["ys"].pop((f, c, hl))
                    dn = rot.tile([1, CH], f32, tag="dn", bufs=1, name="dn")
                    nc.vector.tensor_copy(dn, ys[64:65, :])
                    dv = rot.tile([1, CH], f32, tag="dv", bufs=1, name="dv")
                    nc.vector.reciprocal_approx_fast(dv, dn)
                    bcb = rot.tile([128, CH], f32, tag="bcb", bufs=1,
                                   name="bcb")
                    nc.gpsimd.partition_broadcast(bcb[0:64, :], dv)
                    nc.vector.tensor_mul(
                        att_o[c][hl * 64:(hl + 1) * 64, f, :],
                        ys[0:64, :], bcb[0:64, :])
                return emit

            def attn_steps(c):
                us = []
                for f in range(F):
                    for hl in range(2):
                        for kt in range(4 * (c + 1)):
                            us.append(attn_step1(f, c, kt, hl))
                        us.append(attn_flush1(f, c, hl))
                return us

            # ---------------- phase C: attn(c0) || B(c1) ----------------
            if "C" in PHASES:
                with nc.named_scope("phC"):
                    _interleave(attn_steps(0), b_units(1))

        # ---------------- phases D/E ----------------
        with tc.tile_pool(name="mw", bufs=1) as mw:
            h2 = {}

            def outproj_unit(c, ct):
                def emit():
                    sl = slice(c * CH, (c + 1) * CH)
                    wt = mw.tile([128, F, 128], bf16, tag="woutt", bufs=1,
                                 name="woutt")
                    for kt in range(F):
                        nc.sync.dma_start(
                            wt[:, kt, :],
                            dd["w_out"].ap()[kt * 128:(kt + 1) * 128,
                                             ct * 128:(ct + 1) * 128])
                    ps = ps_aux.tile([128, CH], f32, tag="aux", name="ps_o")
                    for kt in range(F):
                        nc.tensor.matmul(
                            ps, wt[:, kt, :], att_o[c][:, kt, :],
                            start=(kt == 0), stop=(kt == F - 1))
                    t1 = mw.tile([128, CH], f32, tag="t1", bufs=2,
                                 name="o_t1")
                    nc.vector.tensor_scalar_add(t1, ps, bout_c[:, ct:ct + 1])
                    nc.vector.tensor_add(x2_fm[:, ct, sl], t1,
                                         x_fm[c][:, ct, :])
                return emit

            def ln2_unit(c):
                def emit():
                    sl = slice(c * CH, (c + 1) * CH)
                    h2[c] = mw.tile([128, F, CH], bf16, tag="h2", bufs=1,
                                    name=f"h2_{c}")
                    ps_sum = ps_aux.tile([1, CH], f32, tag="aux",
                                         name="ln_sum")
                    for kt in range(F):
                        nc.tensor.matmul(
                            ps_sum, ones_col, x2_fm[:, kt, sl],
                            start=(kt == 0), stop=(kt == F - 1))
                    ps_sq = ps_aux.tile([1, CH], f32, tag="aux",
                                        name="ln_sq")
                    for kt in range(F):
                        sq = mw.tile([128, CH], f32r, tag="t1", bufs=2,
                                     name="sq")
                        nc.vector.tensor_mul(sq, x2_fm[:, kt, sl],
                                             x2_fm[:, kt, sl])
                        nc.tensor.matmul(ps_sq, ones_col, sq,
                                         start=(kt == 0), stop=(kt == F - 1))
                    r_mean = mw.tile([1, CH], f32, tag="lnA", bufs=1,
                                     name="r_mean")
                    nc.vector.tensor_scalar_mul(r_mean, ps_sum, 1.0 / C)
                    r_m2 = mw.tile([1, CH], f32, tag="t1", bufs=2,
                                   name="r_m2")
                    nc.vector.tensor_scalar_mul(r_m2, ps_sq, 1.0 / C)
                    r_msq = mw.tile([1, CH], f32, tag="t1", bufs=2,
                                    name="r_msq")
                    nc.vector.tensor_mul(r_msq, r_mean, r_mean)
                    nc.vector.tensor_sub(r_m2, r_m2, r_msq)
                    nmu_r = mw.tile([1, CH], f32r, tag="lnr_a", bufs=1,
                                    name="nmu_r")
                    nc.vector.tensor_scalar_mul(nmu_r, r_mean, -1.0)
                    # reuse r_mean for ln(var+eps) (mean no longer needed)
                    nc.scalar.activation(r_mean, r_m2, AF.Ln, bias=eps1)
                    rstd_r = mw.tile([1, CH], f32r, tag="lnr_b", bufs=1,
                                     name="rstd_r")
                    nc.scalar.activation(rstd_r, r_mean, AF.Exp, scale=-0.5)
                    bcN = ps_aux.tile([128, CH], f32, tag="aux", name="bcN")
                    nc.tensor.matmul(bcN, ones1, nmu_r, start=True,
                                     stop=True)
                    bcR = ps_aux.tile([128, CH], f32, tag="aux", name="bcR")
                    nc.tensor.matmul(bcR, ones1, rstd_r, start=True,
                                     stop=True)
                    for f in range(F):
                        t1 = mw.tile([128, CH], f32, tag="t1", bufs=2,
                                     name="ln_t1")
                        nc.vector.tensor_add(t1, x2_fm[:, f, sl], bcN)
                        nc.vector.tensor_mul(t1, t1, bcR)
                        nc.vector.tensor_scalar(
                            out=h2[c][:, f, :], in0=t1,
                            scalar1=lnw_c[:, f:f + 1],
                            scalar2=lnb_c[:, f:f + 1],
                            op0=ALU.mult, op1=ALU.add)
                return emit

            g_sb = {}

            def pass1_unit(c, mt):
                def emit():
                    sl = slice(c * CH, (c + 1) * CH)
                    if mt == 0:
                        g_sb[c] = mw.tile([128, MT, CH], bf16, tag="g",
                                          bufs=1, name=f"g{c}")
                    if mt % 3 == 0:
                        wg = mw.tile([128, F, 384], bf16, tag="wc1",
                                     bufs=2, name="wc1g")
                        for kt in range(F):
                            nc.sync.dma_start(
                                wg[:, kt, :],
                                dd["w_c1"].ap()[kt * 128:(kt + 1) * 128,
                                                mt * 128:mt * 128 + 384])
                        st[("wc1g", c)] = wg
                    wg = st[("wc1g", c)]
                    m0 = (mt % 3) * 128
                    ps = ps_acc.tile([128, CH], f32, tag="acc", name="ps_g")
                    for kt in range(F):
                        nc.tensor.matmul(ps, wg[:, kt, m0:m0 + 128],
                                         h2[c][:, kt, :],
                                         start=(kt == 0), stop=(kt == F - 1))
                    # bias added now so the deferred gelu batch is bias-free
                    nc.vector.tensor_scalar_add(
                        g_sb[c][:, mt, :], ps, bc1_c[:, mt:mt + 1])
                return emit

            def gelu_units(c, n_batch):
                us = []
                per = MT // n_batch
                for b in range(n_batch):
                    def emit(b=b):
                        g = g_sb[c]
                        nc.scalar.activation(
                            g[:, b * per:(b + 1) * per, :],
                            g[:, b * per:(b + 1) * per, :], AF.Gelu)
                    us.append(emit)
                return us

            def pass2_unit(c, ct):
                def emit():
                    sl = slice(c * CH, (c + 1) * CH)
                    wt = mw.tile([128, MT, 128], bf16, tag="wc2", bufs=3,
                                 name="wc2t")
                    for mt in range(MT):
                        nc.sync.dma_start(
                            wt[:, mt, :],
                            dd["w_c2"].ap()[mt * 128:(mt + 1) * 128,
                                            ct * 128:(ct + 1) * 128])
                    ps = ps_acc.tile([128, CH], f32, tag="acc", name="ps_m")
                    for mt in range(MT):
                        nc.tensor.matmul(ps, wt[:, mt, :],
                                         g_sb[c][:, mt, :],
                                         start=(mt == 0),
                                         stop=(mt == MT - 1))
                    t1 = mw.tile([128, CH], f32, tag="t1", bufs=2,
                                 name="m_t1")
                    nc.vector.tensor_scalar_add(t1, ps, bc2_c[:, ct:ct + 1])
                    out_fm = st.setdefault(("out", c), None)
                    if out_fm is None:
                        out_fm = rp.tile([128, F, CH], f32, tag=f"xfm{c}",
                                         name=f"out_fm{c}")
                        st[("out", c)] = out_fm
                    nc.vector.tensor_add(out_fm[:, ct, :], t1,
                                         x2_fm[:, ct, sl])
                return emit

            def store_unit(c, tj):
                def emit():
                    t = 4 * c + tj
                    out_fm = st[("out", c)]
                    o_tm = mw.tile([128, C], f32, tag="otm", bufs=1,
                                   name="o_tm")
                    for f in range(F):
                        ps = tr_ps_tile()
                        nc.tensor.transpose(
                            ps, out_fm[:, f, tj * 128:(tj + 1) * 128],
                            ident)
                        nc.vector.tensor_copy(
                            o_tm[:, f * 128:(f + 1) * 128], ps)
                    nc.sync.dma_start(
                        y_d.ap()[t * 128:(t + 1) * 128, :], o_tm)
                return emit

            # ---------------- phase D: attn(c1) || proj/LN2/MLP-p1(c0) ---
            if "D" in PHASES:
                with nc.named_scope("phD"):
                    fillers = ([outproj_unit(0, ct) for ct in range(F)]
                               + [ln2_unit(0)]
                               + [pass1_unit(0, mt) for mt in range(MT)])
                    _interleave(attn_steps(1), fillers)

            # ---------------- phase E ----------------
            if "E" not in PHASES:
                return
            with nc.named_scope("phE1"):
                # ln2(c1) rows use the exp table set; gelu(c0) switches after
                for ct in range(F):
                    outproj_unit(1, ct)()
                ln2_unit(1)()
                for u in gelu_units(0, 6):
                    u()
            with nc.named_scope("phE2"):
                _interleave([pass2_unit(0, ct) for ct in range(F)], [])
            with nc.named_scope("phE3"):
                # pass1(c1) with gelu sub-batches + store(c0) interleaved
                p1 = [pass1_unit(1, mt) for mt in range(MT)]
                fillers = [store_unit(0, tj) for tj in range(4)]
                _interleave(p1, fillers)
                for u in gelu_units(1, 6):
                    u()
            with nc.named_scope("phE4"):
                _interleave([pass2_unit(1, ct) for ct in range(F)], [])
                for tj in range(4):
                    store_unit(1, tj)()


def _get_nc():
    global _NC_CACHE
    if _NC_CACHE is None:
        _NC_CACHE = _build()
    return _NC_CACHE


def _prep_shared(inputs):
    f = lambda k: np.ascontiguousarray(np.asarray(inputs[k], np.float32))
    bf = lambda a: np.ascontiguousarray(a.astype(ml_dtypes.bfloat16))
    w_qkv = f("w_qkv")
    return {
        "w_kq": np.ascontiguousarray(w_qkv[:, :2 * C]),
        "w_v": np.ascontiguousarray(w_qkv[:, 2 * C:]),
        "b_qkv": f("b_qkv"),
        "w_out": bf(f("w_out")),
        "b_out": f("b_out"),
        "w_c1": bf(f("w_c1")),
        "b_c1": f("b_c1"),
        "w_c2": bf(f("w_c2")),
        "b_c2": f("b_c2"),
        "ln_w": f("ln_w"),
        "ln_b": f("ln_b"),
    }


def run(inputs, trace=False):
    nc = _get_nc()
    xs = np.ascontiguousarray(np.asarray(inputs["x"], dtype=np.float32))
    assert xs.shape == (B, T, C), xs.shape
    shared = _prep_shared(inputs)
    in_maps = [dict(shared, x=xs[c]) for c in range(B)]
    res = bass_utils.run_bass_kernel_spmd(
        nc, in_maps, core_ids=list(range(B)), trace=trace)
    out = np.stack([r["y"] for r in res.results], axis=0)
    return out, res


def kernel(**inputs):
    out, _ = run(inputs, trace=False)
    return out


# revision 42
# speedup vs baseline: 1.0898x; 1.0898x over previous
"""Trainium2 Bass kernel for nn_Block_50706383897045 (dense transformer block).

Data-parallel over batch: B=8 == n_cores, one batch element per core, no
collectives. Per core the block runs on a [T=1024, C=768] slice.

v1 restructure (vs the staged baseline): the kernel is emission-interleaved
as a chunk-level software pipeline so the PE never starves (HAM stays at
2.4 GHz):
  A  token-major LN1 on DVE (bn_stats/bn_aggr, no PE stats, no row recips)
     + PE transposes of x and h to feature-major, zipped with chunk-0
     K/Q/V projections.
  C  attention chunk0 zipped with chunk-1 K/Q/V projections.
  D  attention chunk1 zipped with out_proj/LN2/MLP-c1-pass of chunk0.
  E  gelu batches, MLP c2 passes, chunk-1 MLP, stores.
Other changes: causal column restriction in scores/exp/attn.v (-25%),
softmax denominators via batched reciprocal_approx_fast + PE outer-product
broadcast (no 3.3us single-lane DVE recips, no gpsimd broadcasts), rsqrt
via Log/Exp on ACT (stays in the exp table set -> no ACT table thrash),
gelu deferred out of the exp region (2 table loads total), bf16 for
V/out/MLP weights and moving activations (halves SBUF + DMA; PE column
rate is dtype-independent so accuracy is spent only where it buys space).
Scores path (k,q) stays f32r for exp precision.
"""
import os
import sys

sys.path.insert(0, "/opt/trn_rl_repo")

PHASES = os.environ.get("KPH", "ACDE")

import ml_dtypes
import numpy as np

import concourse.bass as bass
import concourse.bacc as bacc
import concourse.mybir as mybir
import concourse.tile as tile
from concourse import bass_utils
from concourse.masks import make_identity

AF = mybir.ActivationFunctionType
ALU = mybir.AluOpType
f32 = mybir.dt.float32
f32r = mybir.dt.float32r
bf16 = mybir.dt.bfloat16

B, T, C, H, D = 8, 1024, 768, 12, 64
F = C // 128      # 6 feature tiles of the residual stream
NT = T // 128     # 8 token tiles
CH = 512          # token chunk
NCH = 2
M3 = 4 * C        # 3072 MLP hidden
MT = M3 // 128    # 24 MLP hidden tiles
EPS = 1e-5

_NC_CACHE = None


def _interleave(steps, fillers):
    """Emit steps and fillers interleaved so both lists finish together."""
    ns, nf = len(steps), len(fillers)
    fi = 0
    for si, s in enumerate(steps):
        s()
        target = (si + 1) * nf // max(ns, 1)
        while fi < target:
            fillers[fi]()
            fi += 1
    while fi < nf:
        fillers[fi]()
        fi += 1


def _build():
    nc = bacc.Bacc("TRN2", target_bir_lowering=False, debug=False,
                   num_devices=8)
    dd = {
        "x": nc.dram_tensor("x", [T, C], f32, kind="ExternalInput"),
        "w_kq": nc.dram_tensor("w_kq", [C, 2 * C], f32, kind="ExternalInput"),
        "w_v": nc.dram_tensor("w_v", [C, C], f32, kind="ExternalInput"),
        "b_qkv": nc.dram_tensor("b_qkv", [3 * C], f32, kind="ExternalInput"),
        "w_out": nc.dram_tensor("w_out", [C, C], bf16, kind="ExternalInput"),
        "b_out": nc.dram_tensor("b_out", [C], f32, kind="ExternalInput"),
        "w_c1": nc.dram_tensor("w_c1", [C, M3], bf16, kind="ExternalInput"),
        "b_c1": nc.dram_tensor("b_c1", [M3], f32, kind="ExternalInput"),
        "w_c2": nc.dram_tensor("w_c2", [M3, C], bf16, kind="ExternalInput"),
        "b_c2": nc.dram_tensor("b_c2", [C], f32, kind="ExternalInput"),
        "ln_w": nc.dram_tensor("ln_w", [C], f32, kind="ExternalInput"),
        "ln_b": nc.dram_tensor("ln_b", [C], f32, kind="ExternalInput"),
        "y": nc.dram_tensor("y", [T, C], f32, kind="ExternalOutput"),
    }
    with tile.TileContext(nc) as tc:
        _body(nc, tc, dd)
    nc.compile()
    return nc


def _col_rearr(ap, p=128):
    return ap.rearrange("(o p) -> p o", p=p)


def _body(nc, tc, dd):
    x_d, y_d = dd["x"], dd["y"]
    with tc.tile_pool(name="persist", bufs=1) as pp:
        ident = pp.tile([128, 128], f32, name="ident")
        make_identity(nc, ident)
        ones1 = pp.tile([1, 128], f32r, name="ones1")
        nc.vector.memset(ones1.bitcast(f32), 1.0)
        ones_col = pp.tile([128, 1], f32r, name="ones_col")
        nc.vector.memset(ones_col.bitcast(f32), 1.0)

        eps128 = pp.tile([128, 1], f32, name="eps128")
        nc.vector.memset(eps128, EPS)
        eps1 = pp.tile([1, 1], f32, name="eps1")
        nc.vector.memset(eps1, EPS)
        lnw_c = pp.tile([128, F], f32, name="lnw_c")
        nc.sync.dma_start(lnw_c, _col_rearr(dd["ln_w"].ap()))
        lnb_c = pp.tile([128, F], f32, name="lnb_c")
        nc.sync.dma_start(lnb_c, _col_rearr(dd["ln_b"].ap()))
        bkq_c = pp.tile([128, 12], f32, name="bkq_c")
        nc.sync.dma_start(bkq_c, _col_rearr(dd["b_qkv"].ap()[0:2 * C]))
        bout_c = pp.tile([128, F], f32, name="bout_c")
        nc.sync.dma_start(bout_c, _col_rearr(dd["b_out"].ap()))
        bc1_c = pp.tile([128, MT], f32, name="bc1_c")
        nc.sync.dma_start(bc1_c, _col_rearr(dd["b_c1"].ap()))
        bc2_c = pp.tile([128, F], f32, name="bc2_c")
        nc.sync.dma_start(bc2_c, _col_rearr(dd["b_c2"].ap()))
        # V bias broadcast along partitions: [128, C]
        bv_bc = pp.tile([128, C], f32, name="bv_bc")
        bv_src = dd["b_qkv"].ap()[2 * C:3 * C]
        bv_b = bass.AP(tensor=bv_src.tensor, offset=bv_src.offset,
                       ap=[[0, 128]] + [list(p) for p in bv_src.ap])
        nc.gpsimd.dma_start(out=bv_bc, in_=bv_b)
        # single causal 0/1 mask (keep p <= j); offset masks are its
        # column-shifted slices
        masks = pp.tile([128, CH], bf16, name="masks")

        _main(nc, tc, dd, ident, ones1, ones_col, eps128,
              eps1, lnw_c, lnb_c, bkq_c, bout_c, bc1_c, bc2_c, bv_bc, masks)


def _main(nc, tc, dd, ident, ones1, ones_col, eps128, eps1,
          lnw_c, lnb_c, bkq_c, bout_c, bc1_c, bc2_c, bv_bc, masks):
    x_d, y_d = dd["x"], dd["y"]
    with (
        tc.tile_pool(name="resid", bufs=1) as rp,
        tc.tile_pool(name="attst", bufs=1) as asp,
        tc.tile_pool(name="rot", bufs=1) as rot,
        tc.tile_pool(name="ps_acc", bufs=2, space="PSUM") as ps_acc,
        tc.tile_pool(name="ps_s", bufs=2, space="PSUM") as ps_s,
        tc.tile_pool(name="ps_y", bufs=2, space="PSUM") as ps_y,
        tc.tile_pool(name="ps_aux", bufs=2, space="PSUM") as ps_aux,
    ):
        x_fm = [rp.tile([128, F, CH], f32, tag=f"xfm{c}", name=f"x_fm{c}")
                for c in range(NCH)]
        x2_fm = rp.tile([128, F, T], f32r, name="x2_fm")
        kf = asp.tile([128, F, T], f32r, name="kf")
        v1 = asp.tile([128, NT, H * 65], bf16, name="v1")
        att_o = [asp.tile([128, F, CH], bf16, tag=f"ao{c}", name=f"ao{c}")
                 for c in range(NCH)]
        nc.vector.memset(
            v1.rearrange("p t (h m) -> p t h m", m=65)[:, :, :, 64:65], 1.0)

        st = {"qf": {}, "pend": {}, "ys": {}, "trslot": [0]}

        def tr_ps_tile(shape=(128, 128)):
            # rotate transpose/aux PSUM slots across the s/y/aux pools
            i = st["trslot"][0]
            st["trslot"][0] = (i + 1) % 3
            pool = (ps_s, ps_y, ps_aux)[i]
            tag = ("s", "y", "aux")[i]
            return pool.tile(list(shape), f32, tag=tag, name="tr")

        # ---------------- phase A + B(c0) ----------------
        with tc.tile_pool(name="fw", bufs=1) as fw:
            wv_t = []
            for kt in range(F):
                wt = fw.tile([128, C], f32r, tag=f"wv{kt}", name=f"wv{kt}")
                nc.sync.dma_start(
                    wt, dd["w_v"].ap().bitcast(f32r)
                    [kt * 128:(kt + 1) * 128, :])
                wv_t.append(wt)
            h_fm = fw.tile([128, F, T], f32r, name="h_fm")

            def v_unit(c, t, half):
                def emit():
                    ps = ps_acc.tile([128, 384], f32, tag="acc", name="ps_v")
                    c0 = half * 384
                    for kt in range(F):
                        nc.tensor.matmul(
                            ps, h_fm[:, kt, t * 128:(t + 1) * 128],
                            wv_t[kt][:, c0:c0 + 384],
                            start=(kt == 0), stop=(kt == F - 1))
                    dst = (v1[:, t, :].rearrange("p (h m) -> p h m", m=65)
                           [:, half * 6:(half + 1) * 6, 0:64])
                    src = ps.rearrange("p (h m) -> p h m", m=64)
                    bias = (bv_bc[:, c0:c0 + 384]
                            .rearrange("p (h m) -> p h m", m=64))
                    nc.vector.tensor_add(dst, src, bias)
                return emit

            def kq_unit(c, f, which):
                def emit():
                    sl = slice(c * CH, (c + 1) * CH)
                    col0 = which * C + f * 128
                    wt = fw.tile([128, F, 128], f32r, tag="wkqt", bufs=2,
                                 name="wkqt")
                    for kt in range(F):
                        nc.sync.dma_start(
                            wt[:, kt, :],
                            dd["w_kq"].ap().bitcast(f32r)
                            [kt * 128:(kt + 1) * 128, col0:col0 + 128])
                    ps = ps_acc.tile([128, CH], f32, tag="acc", name="ps_kq")
                    for kt in range(F):
                        nc.tensor.matmul(
                            ps, wt[:, kt, :], h_fm[:, kt, sl],
                            start=(kt == 0), stop=(kt == F - 1))
                    if which == 0:
                        nc.vector.tensor_scalar_add(
                            kf[:, f, sl], ps, bkq_c[:, f:f + 1])
                    else:
                        qt = rot.tile([128, CH], f32r, tag=f"qf{c}",
                                      bufs=(4 if c == 0 else 6),
                                      name=f"qf{f}_{c}")
                        nc.vector.tensor_scalar_add(
                            qt, ps, bkq_c[:, F + f:F + f + 1])
                        st["qf"][(f, c)] = qt
                return emit

            def b_units(c):
                us = []
                for t in range(4 * c, 4 * (c + 1)):
                    for half in range(2):
                        us.append(v_unit(c, t, half))
                for f in range(F):
                    us.append(kq_unit(c, f, 0))
                    us.append(kq_unit(c, f, 1))
                return us

            with tc.tile_pool(name="ft", bufs=1) as ft:
                # build the bf16 causal mask via a small f32 temp
                mtmp = ft.tile([128, CH], f32, tag="htm", bufs=2,
                               name="mtmp")
                nc.vector.memset(mtmp, 1.0)
                nc.gpsimd.affine_select(
                    out=mtmp, in_=mtmp,
                    compare_op=ALU.is_ge, fill=0.0,
                    base=0, pattern=[[1, CH]],
                    channel_multiplier=-1)
                nc.vector.tensor_copy(masks, mtmp)

                def a_unit(t):
                    def emit():
                        x_tm = ft.tile([128, C], f32, tag="xtm", bufs=2,
                                       name=f"x_tm{t}")
                        nc.sync.dma_start(
                            x_tm, x_d.ap()[t * 128:(t + 1) * 128, :])
                        bst = ft.tile([128, 3, 6], f32, tag="bnst", bufs=2,
                                      name="bst")
                        for g in range(3):
                            nc.vector.bn_stats(
                                bst[:, g, :], x_tm[:, g * 256:(g + 1) * 256])
                        mv = ft.tile([128, 2], f32, tag="mv", bufs=2,
                                     name="mv")
                        nc.vector.bn_aggr(mv, bst)
                        lv = ft.tile([128, 1], f32, tag="lv", bufs=2,
                                     name="lv")
                        nc.scalar.activation(lv, mv[:, 1:2], AF.Ln,
                                             bias=eps128)
                        rstd = ft.tile([128, 1], f32, tag="rstd", bufs=2,
                                       name="rstd")
                        nc.scalar.activation(rstd, lv, AF.Exp, scale=-0.5)
                        negmu = ft.tile([128, 1], f32, tag="negmu", bufs=2,
                                        name="negmu")
                        nc.vector.tensor_scalar_mul(negmu, mv[:, 0:1], -1.0)
                        h_tm = ft.tile([128, C], f32, tag="htm", bufs=2,
                                       name="h_tm")
                        nc.vector.tensor_scalar(
                            out=h_tm, in0=x_tm, scalar1=negmu, scalar2=rstd,
                            op0=ALU.add, op1=ALU.mult)
                        c, tj = t // 4, t % 4
                        for f in range(F):
                            fs = slice(f * 128, (f + 1) * 128)
                            ps1 = tr_ps_tile()
                            nc.tensor.transpose(ps1, x_tm[:, fs], ident)
                            nc.scalar.activation(
                                x_fm[c][:, f, tj * 128:(tj + 1) * 128],
                                ps1, AF.Copy)
                            ps2 = tr_ps_tile()
                            nc.tensor.transpose(ps2, h_tm[:, fs], ident)
                            nc.vector.tensor_scalar(
                                out=h_fm[:, f, t * 128:(t + 1) * 128],
                                in0=ps2, scalar1=lnw_c[:, f:f + 1],
                                scalar2=lnb_c[:, f:f + 1],
                                op0=ALU.mult, op1=ALU.add)
                    return emit

                with nc.named_scope("phA"):
                    for t in range(4):
                        a_unit(t)()
                    _interleave([a_unit(t) for t in range(4, 8)],
                                b_units(0))

            # ---------------- attention machinery ----------------
            # NOTE: heads run sequentially; alternating base-0/base-64
            # scores matmuls back-to-back silently corrupts the PE output
            # on this hardware (row-group concurrency issue).
            def _attnv1(f, c, hl, kt, e, sub):
                ktmax = 4 * (c + 1)
                h = 2 * f + hl
                nc.tensor.matmul(
                    st["ys"][(f, c, hl)][:, sub],
                    v1[:, kt, h * 65:(h + 1) * 65], e[:, sub],
                    start=(kt == 0), stop=(kt == ktmax - 1))

            def attn_step1(f, c, kt, hl):
                def emit():
                    off = c * CH - kt * 128
                    lo = max(0, -off)
                    sub = slice(lo, CH)
                    qt = st["qf"][(f, c)]
                    if kt == 0:
                        st["ys"][(f, c, hl)] = ps_y.tile(
                            [65, CH], f32, tag="y", name=f"y{f}_{c}_{hl}")
                    ps = ps_s.tile([128, CH], f32, tag="s", name="s")
                    nc.tensor.matmul(
                        ps[:, sub],
                        kf[hl * 64:(hl + 1) * 64, f,
                           kt * 128:(kt + 1) * 128],
                        qt[hl * 64:(hl + 1) * 64, sub],
                        start=True, stop=True)
                    e = rot.tile([128, CH], bf16, tag="e", bufs=4,
                                 name="expt")
                    nc.scalar.activation(e[:, sub], ps[:, sub], AF.Exp)
                    if off < 128:
                        nc.vector.tensor_mul(
                            e[:, sub], e[:, sub], masks[:, 0:CH - lo])
                    prev = st["pend"].pop((f, c, hl), None)
                    if prev is not None:
                        _attnv1(f, c, hl, *prev)
                    st["pend"][(f, c, hl)] = (kt, e, sub)
                return emit

            def attn_flush1(f, c, hl):
                def emit():
                    prev = st["pend"].pop((f, c, hl))
                    _attnv1(f, c, hl, *prev)
                    ys = st["ys"].pop((f, c, hl))
                    dn = rot.tile([1, CH], f32, tag="dn", bufs=1, name="dn")
                    nc.vector.tensor_copy(dn, ys[64:65, :])
                    dv = rot.tile([1, CH], f32, tag="dv", bufs=1, name="dv")
                    nc.vector.reciprocal_approx_fast(dv, dn)
                    bcb = rot.tile([128, CH], f32, tag="bcb", bufs=1,
                                   name="bcb")
                    nc.gpsimd.partition_broadcast(bcb[0:64, :], dv)
                    nc.vector.tensor_mul(
                        att_o[c][hl * 64:(hl + 1) * 64, f, :],
                        ys[0:64, :], bcb[0:64, :])
                return emit

            def attn_steps(c):
                us = []
                for f in range(F):
                    for hl in range(2):
                        for kt in range(4 * (c + 1)):
                            us.append(attn_step1(f, c, kt, hl))
                        us.append(attn_flush1(f, c, hl))
                return us

            # ---------------- phase C: attn(c0) || B(c1) ----------------
            if "C" in PHASES:
                with nc.named_scope("phC"):
                    _interleave(attn_steps(0), b_units(1))

        # ---------------- phases D/E ----------------
        with tc.tile_pool(name="mw", bufs=1) as mw:
            h2 = {}

            def outproj_unit(c, ct):
                def emit():
                    sl = slice(c * CH, (c + 1) * CH)
                    wt = mw.tile([128, F, 128], bf16, tag="woutt", bufs=2,
                                 name="woutt")
                    for kt in range(F):
                        nc.sync.dma_start(
                            wt[:, kt, :],
                            dd["w_out"].ap()[kt * 128:(kt + 1) * 128,
                                             ct * 128:(ct + 1) * 128])
                    ps = ps_aux.tile([128, CH], f32, tag="aux", name="ps_o")
                    for kt in range(F):
                        nc.tensor.matmul(
                            ps, wt[:, kt, :], att_o[c][:, kt, :],
                            start=(kt == 0), stop=(kt == F - 1))
                    t1 = mw.tile([128, CH], f32, tag="t1", bufs=2,
                                 name="o_t1")
                    nc.vector.tensor_scalar_add(t1, ps, bout_c[:, ct:ct + 1])
                    nc.vector.tensor_add(x2_fm[:, ct, sl], t1,
                                         x_fm[c][:, ct, :])
                return emit

            def ln2_unit(c):
                def emit():
                    sl = slice(c * CH, (c + 1) * CH)
                    h2[c] = mw.tile([128, F, CH], bf16, tag="h2", bufs=1,
                                    name=f"h2_{c}")
                    ps_sum = ps_aux.tile([1, CH], f32, tag="aux",
                                         name="ln_sum")
                    for kt in range(F):
                        nc.tensor.matmul(
                            ps_sum, ones_col, x2_fm[:, kt, sl],
                            start=(kt == 0), stop=(kt == F - 1))
                    ps_sq = ps_aux.tile([1, CH], f32, tag="aux",
                                        name="ln_sq")
                    for kt in range(F):
                        sq = mw.tile([128, CH], f32r, tag="t1", bufs=2,
                                     name="sq")
                        nc.vector.tensor_mul(sq, x2_fm[:, kt, sl],
                                             x2_fm[:, kt, sl])
                        nc.tensor.matmul(ps_sq, ones_col, sq,
                                         start=(kt == 0), stop=(kt == F - 1))
                    r_mean = mw.tile([1, CH], f32, tag="lnA", bufs=1,
                                     name="r_mean")
                    nc.vector.tensor_scalar_mul(r_mean, ps_sum, 1.0 / C)
                    r_m2 = mw.tile([1, CH], f32, tag="t1", bufs=2,
                                   name="r_m2")
                    nc.vector.tensor_scalar_mul(r_m2, ps_sq, 1.0 / C)
                    r_msq = mw.tile([1, CH], f32, tag="t1", bufs=2,
                                    name="r_msq")
                    nc.vector.tensor_mul(r_msq, r_mean, r_mean)
                    nc.vector.tensor_sub(r_m2, r_m2, r_msq)
                    nmu_r = mw.tile([1, CH], f32r, tag="lnr_a", bufs=1,
                                    name="nmu_r")
                    nc.vector.tensor_scalar_mul(nmu_r, r_mean, -1.0)
                    # reuse r_mean for ln(var+eps) (mean no longer needed)
                    nc.scalar.activation(r_mean, r_m2, AF.Ln, bias=eps1)
                    rstd_r = mw.tile([1, CH], f32r, tag="lnr_b", bufs=1,
                                     name="rstd_r")
                    nc.scalar.activation(rstd_r, r_mean, AF.Exp, scale=-0.5)
                    bcN = ps_aux.tile([128, CH], f32, tag="aux", name="bcN")
                    nc.tensor.matmul(bcN, ones1, nmu_r, start=True,
                                     stop=True)
                    bcR = ps_aux.tile([128, CH], f32, tag="aux", name="bcR")
                    nc.tensor.matmul(bcR, ones1, rstd_r, start=True,
                                     stop=True)
                    for f in range(F):
                        t1 = mw.tile([128, CH], f32, tag="t1", bufs=2,
                                     name="ln_t1")
                        nc.vector.tensor_add(t1, x2_fm[:, f, sl], bcN)
                        nc.vector.tensor_mul(t1, t1, bcR)
                        nc.vector.tensor_scalar(
                            out=h2[c][:, f, :], in0=t1,
                            scalar1=lnw_c[:, f:f + 1],
                            scalar2=lnb_c[:, f:f + 1],
                            op0=ALU.mult, op1=ALU.add)
                return emit

            g_sb = {}

            def pass1_unit(c, mt):
                def emit():
                    sl = slice(c * CH, (c + 1) * CH)
                    if mt == 0:
                        g_sb[c] = mw.tile([128, MT, CH], bf16, tag="g",
                                          bufs=1, name=f"g{c}")
                    if mt % 4 == 0:
                        wg = mw.tile([128, F, 512], bf16, tag="wc1",
                                     bufs=2, name="wc1g")
                        for kt in range(F):
                            nc.sync.dma_start(
                                wg[:, kt, :],
                                dd["w_c1"].ap()[kt * 128:(kt + 1) * 128,
                                                mt * 128:mt * 128 + 512])
                        st[("wc1g", c)] = wg
                    wg = st[("wc1g", c)]
                    m0 = (mt % 4) * 128
                    ps = ps_acc.tile([128, CH], f32, tag="acc", name="ps_g")
                    for kt in range(F):
                        nc.tensor.matmul(ps, wg[:, kt, m0:m0 + 128],
                                         h2[c][:, kt, :],
                                         start=(kt == 0), stop=(kt == F - 1))
                    # bias added now so the deferred gelu batch is bias-free
                    nc.vector.tensor_scalar_add(
                        g_sb[c][:, mt, :], ps, bc1_c[:, mt:mt + 1])
                return emit

            def gelu_units(c, n_batch):
                us = []
                per = MT // n_batch
                for b in range(n_batch):
                    def emit(b=b):
                        g = g_sb[c]
                        nc.scalar.activation(
                            g[:, b * per:(b + 1) * per, :],
                            g[:, b * per:(b + 1) * per, :], AF.Gelu)
                    us.append(emit)
                return us

            def pass2_unit(c, ct):
                def emit():
                    sl = slice(c * CH, (c + 1) * CH)
                    wt = mw.tile([128, MT, 128], bf16, tag="wc2", bufs=3,
                                 name="wc2t")
                    for mt in range(MT):
                        nc.sync.dma_start(
                            wt[:, mt, :],
                            dd["w_c2"].ap()[mt * 128:(mt + 1) * 128,
                                            ct * 128:(ct + 1) * 128])
                    ps = ps_acc.tile([128, CH], f32, tag="acc", name="ps_m")
                    for mt in range(MT):
                        nc.tensor.matmul(ps, wt[:, mt, :],
                                         g_sb[c][:, mt, :],
                                         start=(mt == 0),
                                         stop=(mt == MT - 1))
                    t1 = mw.tile([128, CH], f32, tag="t1", bufs=2,
                                 name="m_t1")
                    nc.vector.tensor_scalar_add(t1, ps, bc2_c[:, ct:ct + 1])
                    out_fm = st.setdefault(("out", c), None)
                    if out_fm is None:
                        out_fm = rp.tile([128, F, CH], f32, tag=f"xfm{c}",
                                         name=f"out_fm{c}")
                        st[("out", c)] = out_fm
                    nc.vector.tensor_add(out_fm[:, ct, :], t1,
                                         x2_fm[:, ct, sl])
                return emit

            def store_unit(c, tj):
                def emit():
                    t = 4 * c + tj
                    out_fm = st[("out", c)]
                    o_tm = mw.tile([128, C], f32, tag="otm", bufs=1,
                                   name="o_tm")
                    for f in range(F):
                        ps = tr_ps_tile()
                        nc.tensor.transpose(
                            ps, out_fm[:, f, tj * 128:(tj + 1) * 128],
                            ident)
                        nc.vector.tensor_copy(
                            o_tm[:, f * 128:(f + 1) * 128], ps)
                    nc.sync.dma_start(
                        y_d.ap()[t * 128:(t + 1) * 128, :], o_tm)
                return emit

            # ---------------- phase D: attn(c1) || proj/LN2/MLP-p1(c0) ---
            if "D" in PHASES:
                with nc.named_scope("phD"):
                    fillers = ([outproj_unit(0, ct) for ct in range(F)]
                               + [ln2_unit(0)]
                               + [pass1_unit(0, mt) for mt in range(MT)])
                    _interleave(attn_steps(1), fillers)

            # ---------------- phase E ----------------
            if "E" not in PHASES:
                return
            with nc.named_scope("phE1"):
                # ln2(c1) rows use the exp table set; gelu(c0) switches after
                for ct in range(F):
                    outproj_unit(1, ct)()
                ln2_unit(1)()
                for u in gelu_units(0, 6):
                    u()
            with nc.named_scope("phE2"):
                _interleave([pass2_unit(0, ct) for ct in range(F)], [])
            with nc.named_scope("phE3"):
                # pass1(c1) with gelu sub-batches + store(c0) interleaved
                p1 = [pass1_unit(1, mt) for mt in range(MT)]
                fillers = [store_unit(0, tj) for tj in range(4)]
                _interleave(p1, fillers)
                for u in gelu_units(1, 6):
                    u()
            with nc.named_scope("phE4"):
                _interleave([pass2_unit(1, ct) for ct in range(F)], [])
                for tj in range(4):
                    store_unit(1, tj)()


def _get_nc():
    global _NC_CACHE
    if _NC_CACHE is None:
        _NC_CACHE = _build()
    return _NC_CACHE


def _prep_shared(inputs):
    f = lambda k: np.ascontiguousarray(np.asarray(inputs[k], np.float32))
    bf = lambda a: np.ascontiguousarray(a.astype(ml_dtypes.bfloat16))
    w_qkv = f("w_qkv")
    return {
        "w_kq": np.ascontiguousarray(w_qkv[:, :2 * C]),
        "w_v": np.ascontiguousarray(w_qkv[:, 2 * C:]),
        "b_qkv": f("b_qkv"),
        "w_out": bf(f("w_out")),
        "b_out": f("b_out"),
        "w_c1": bf(f("w_c1")),
        "b_c1": f("b_c1"),
        "w_c2": bf(f("w_c2")),
        "b_c2": f("b_c2"),
        "ln_w": f("ln_w"),
        "ln_b": f("ln_b"),
    }


def run(inputs, trace=False):
    nc = _get_nc()
    xs = np.ascontiguousarray(np.asarray(inputs["x"], dtype=np.float32))
    assert xs.shape == (B, T, C), xs.shape
    shared = _prep_shared(inputs)
    in_maps = [dict(shared, x=xs[c]) for c in range(B)]
    res = bass_utils.run_bass_kernel_spmd(
        nc, in_maps, core_ids=list(range(B)), trace=trace)
    out = np.stack([r["y"] for r in res.results], axis=0)
    return out, res


def kernel(**inputs):
    out, _ = run(inputs, trace=False)
    return out


# revision 43
# speedup vs baseline: 1.2142x; 1.1142x over previous
"""Trainium2 Bass kernel for nn_Block_50706383897045 (dense transformer block).

Data-parallel over batch: B=8 == n_cores, one batch element per core, no
collectives. Per core the block runs on a [T=1024, C=768] slice.

v1 restructure (vs the staged baseline): the kernel is emission-interleaved
as a chunk-level software pipeline so the PE never starves (HAM stays at
2.4 GHz):
  A  token-major LN1 on DVE (bn_stats/bn_aggr, no PE stats, no row recips)
     + PE transposes of x and h to feature-major, zipped with chunk-0
     K/Q/V projections.
  C  attention chunk0 zipped with chunk-1 K/Q/V projections.
  D  attention chunk1 zipped with out_proj/LN2/MLP-c1-pass of chunk0.
  E  gelu batches, MLP c2 passes, chunk-1 MLP, stores.
Other changes: causal column restriction in scores/exp/attn.v (-25%),
softmax denominators via batched reciprocal_approx_fast + PE outer-product
broadcast (no 3.3us single-lane DVE recips, no gpsimd broadcasts), rsqrt
via Log/Exp on ACT (stays in the exp table set -> no ACT table thrash),
gelu deferred out of the exp region (2 table loads total), bf16 for
V/out/MLP weights and moving activations (halves SBUF + DMA; PE column
rate is dtype-independent so accuracy is spent only where it buys space).
Scores path (k,q) stays f32r for exp precision.
"""
import os
import sys

sys.path.insert(0, "/opt/trn_rl_repo")

PHASES = os.environ.get("KPH", "ACDE")

import ml_dtypes
import numpy as np

import concourse.bass as bass
import concourse.bacc as bacc
import concourse.mybir as mybir
import concourse.tile as tile
from concourse import bass_utils
from concourse.masks import make_identity

AF = mybir.ActivationFunctionType
ALU = mybir.AluOpType
f32 = mybir.dt.float32
f32r = mybir.dt.float32r
bf16 = mybir.dt.bfloat16

B, T, C, H, D = 8, 1024, 768, 12, 64
F = C // 128      # 6 feature tiles of the residual stream
NT = T // 128     # 8 token tiles
CH = 512          # token chunk
NCH = 2
M3 = 4 * C        # 3072 MLP hidden
MT = M3 // 128    # 24 MLP hidden tiles
EPS = 1e-5

_NC_CACHE = None


def _interleave(steps, fillers):
    """Emit steps and fillers interleaved so both lists finish together."""
    ns, nf = len(steps), len(fillers)
    fi = 0
    for si, s in enumerate(steps):
        s()
        target = (si + 1) * nf // max(ns, 1)
        while fi < target:
            fillers[fi]()
            fi += 1
    while fi < nf:
        fillers[fi]()
        fi += 1


def _build():
    nc = bacc.Bacc("TRN2", target_bir_lowering=False, debug=False,
                   num_devices=8)
    dd = {
        "x": nc.dram_tensor("x", [T, C], f32, kind="ExternalInput"),
        "w_kq": nc.dram_tensor("w_kq", [C, 2 * C], f32, kind="ExternalInput"),
        "w_v": nc.dram_tensor("w_v", [C, C], f32, kind="ExternalInput"),
        "b_qkv": nc.dram_tensor("b_qkv", [3 * C], f32, kind="ExternalInput"),
        "w_out": nc.dram_tensor("w_out", [C, C], bf16, kind="ExternalInput"),
        "b_out": nc.dram_tensor("b_out", [C], f32, kind="ExternalInput"),
        "w_c1": nc.dram_tensor("w_c1", [C, M3], bf16, kind="ExternalInput"),
        "b_c1": nc.dram_tensor("b_c1", [M3], f32, kind="ExternalInput"),
        "w_c2": nc.dram_tensor("w_c2", [M3, C], bf16, kind="ExternalInput"),
        "b_c2": nc.dram_tensor("b_c2", [C], f32, kind="ExternalInput"),
        "ln_w": nc.dram_tensor("ln_w", [C], f32, kind="ExternalInput"),
        "ln_b": nc.dram_tensor("ln_b", [C], f32, kind="ExternalInput"),
        "y": nc.dram_tensor("y", [T, C], f32, kind="ExternalOutput"),
    }
    with tile.TileContext(nc) as tc:
        _body(nc, tc, dd)
    nc.compile()
    return nc


def _col_rearr(ap, p=128):
    return ap.rearrange("(o p) -> p o", p=p)


def _body(nc, tc, dd):
    x_d, y_d = dd["x"], dd["y"]
    with tc.tile_pool(name="persist", bufs=1) as pp:
        ident = pp.tile([128, 128], f32, name="ident")
        make_identity(nc, ident)
        ones1 = pp.tile([1, 128], f32r, name="ones1")
        nc.vector.memset(ones1.bitcast(f32), 1.0)
        ones_col = pp.tile([128, 1], f32r, name="ones_col")
        nc.vector.memset(ones_col.bitcast(f32), 1.0)

        eps128 = pp.tile([128, 1], f32, name="eps128")
        nc.vector.memset(eps128, EPS)
        eps1 = pp.tile([1, 1], f32, name="eps1")
        nc.vector.memset(eps1, EPS)
        lnw_c = pp.tile([128, F], f32, name="lnw_c")
        nc.sync.dma_start(lnw_c, _col_rearr(dd["ln_w"].ap()))
        lnb_c = pp.tile([128, F], f32, name="lnb_c")
        nc.sync.dma_start(lnb_c, _col_rearr(dd["ln_b"].ap()))
        bkq_c = pp.tile([128, 12], f32, name="bkq_c")
        nc.sync.dma_start(bkq_c, _col_rearr(dd["b_qkv"].ap()[0:2 * C]))
        bout_c = pp.tile([128, F], f32, name="bout_c")
        nc.sync.dma_start(bout_c, _col_rearr(dd["b_out"].ap()))
        bc1_c = pp.tile([128, MT], f32, name="bc1_c")
        nc.sync.dma_start(bc1_c, _col_rearr(dd["b_c1"].ap()))
        bc2_c = pp.tile([128, F], f32, name="bc2_c")
        nc.sync.dma_start(bc2_c, _col_rearr(dd["b_c2"].ap()))
        # V bias broadcast along partitions: [128, C]
        bv_bc = pp.tile([128, C], f32, name="bv_bc")
        bv_src = dd["b_qkv"].ap()[2 * C:3 * C]
        bv_b = bass.AP(tensor=bv_src.tensor, offset=bv_src.offset,
                       ap=[[0, 128]] + [list(p) for p in bv_src.ap])
        nc.gpsimd.dma_start(out=bv_bc, in_=bv_b)
        # single causal 0/1 mask (keep p <= j); offset masks are its
        # column-shifted slices
        masks = pp.tile([128, CH], bf16, name="masks")

        _main(nc, tc, dd, ident, ones1, ones_col, eps128,
              eps1, lnw_c, lnb_c, bkq_c, bout_c, bc1_c, bc2_c, bv_bc, masks)


def _main(nc, tc, dd, ident, ones1, ones_col, eps128, eps1,
          lnw_c, lnb_c, bkq_c, bout_c, bc1_c, bc2_c, bv_bc, masks):
    x_d, y_d = dd["x"], dd["y"]
    with (
        tc.tile_pool(name="resid", bufs=1) as rp,
        tc.tile_pool(name="attst", bufs=1) as asp,
        tc.tile_pool(name="rot", bufs=1) as rot,
        tc.tile_pool(name="ps_acc", bufs=2, space="PSUM") as ps_acc,
        tc.tile_pool(name="ps_s", bufs=2, space="PSUM") as ps_s,
        tc.tile_pool(name="ps_y", bufs=2, space="PSUM") as ps_y,
        tc.tile_pool(name="ps_aux", bufs=2, space="PSUM") as ps_aux,
    ):
        x_fm = [rp.tile([128, F, CH], f32, tag=f"xfm{c}", name=f"x_fm{c}")
                for c in range(NCH)]
        x2_fm = rp.tile([128, F, T], f32r, name="x2_fm")
        kf = asp.tile([128, F, T], f32r, name="kf")
        v1 = asp.tile([128, NT, H * 65], bf16, name="v1")
        att_o = [asp.tile([128, F, CH], bf16, tag=f"ao{c}", name=f"ao{c}")
                 for c in range(NCH)]
        nc.vector.memset(
            v1.rearrange("p t (h m) -> p t h m", m=65)[:, :, :, 64:65], 1.0)

        st = {"qf": {}, "pend": {}, "ys": {}, "trslot": [0]}

        def tr_ps_tile(shape=(128, 128)):
            # rotate transpose/aux PSUM slots across the s/y/aux pools
            i = st["trslot"][0]
            st["trslot"][0] = (i + 1) % 3
            pool = (ps_s, ps_y, ps_aux)[i]
            tag = ("s", "y", "aux")[i]
            return pool.tile(list(shape), f32, tag=tag, name="tr")

        # ---------------- phase A + B(c0) ----------------
        with tc.tile_pool(name="fw", bufs=1) as fw:
            wv_t = []
            for kt in range(F):
                wt = fw.tile([128, C], f32r, tag=f"wv{kt}", name=f"wv{kt}")
                nc.sync.dma_start(
                    wt, dd["w_v"].ap().bitcast(f32r)
                    [kt * 128:(kt + 1) * 128, :])
                wv_t.append(wt)
            h_fm = fw.tile([128, F, T], f32r, name="h_fm")

            def v_unit(c, t, half):
                def emit():
                    ps = ps_acc.tile([128, 384], f32, tag="acc", name="ps_v")
                    c0 = half * 384
                    for kt in range(F):
                        nc.tensor.matmul(
                            ps, h_fm[:, kt, t * 128:(t + 1) * 128],
                            wv_t[kt][:, c0:c0 + 384],
                            start=(kt == 0), stop=(kt == F - 1))
                    dst = (v1[:, t, :].rearrange("p (h m) -> p h m", m=65)
                           [:, half * 6:(half + 1) * 6, 0:64])
                    src = ps.rearrange("p (h m) -> p h m", m=64)
                    bias = (bv_bc[:, c0:c0 + 384]
                            .rearrange("p (h m) -> p h m", m=64))
                    nc.vector.tensor_add(dst, src, bias)
                return emit

            def kq_unit(c, f, which):
                def emit():
                    sl = slice(c * CH, (c + 1) * CH)
                    col0 = which * C + f * 128
                    wt = fw.tile([128, F, 128], f32r, tag="wkqt", bufs=2,
                                 name="wkqt")
                    for kt in range(F):
                        nc.sync.dma_start(
                            wt[:, kt, :],
                            dd["w_kq"].ap().bitcast(f32r)
                            [kt * 128:(kt + 1) * 128, col0:col0 + 128])
                    ps = ps_acc.tile([128, CH], f32, tag="acc", name="ps_kq")
                    for kt in range(F):
                        nc.tensor.matmul(
                            ps, wt[:, kt, :], h_fm[:, kt, sl],
                            start=(kt == 0), stop=(kt == F - 1))
                    if which == 0:
                        nc.vector.tensor_scalar_add(
                            kf[:, f, sl], ps, bkq_c[:, f:f + 1])
                    else:
                        qt = rot.tile([128, CH], f32r, tag=f"qf{c}",
                                      bufs=(5 if c == 0 else 6),
                                      name=f"qf{f}_{c}")
                        nc.vector.tensor_scalar_add(
                            qt, ps, bkq_c[:, F + f:F + f + 1])
                        st["qf"][(f, c)] = qt
                return emit

            def b_units(c):
                us = []
                for t in range(4 * c, 4 * (c + 1)):
                    for half in range(2):
                        us.append(v_unit(c, t, half))
                for f in range(F):
                    us.append(kq_unit(c, f, 0))
                    us.append(kq_unit(c, f, 1))
                return us

            with tc.tile_pool(name="ft", bufs=1) as ft:
                # build the bf16 causal mask via a small f32 temp
                mtmp = ft.tile([128, CH], f32, tag="htm", bufs=2,
                               name="mtmp")
                nc.vector.memset(mtmp, 1.0)
                nc.gpsimd.affine_select(
                    out=mtmp, in_=mtmp,
                    compare_op=ALU.is_ge, fill=0.0,
                    base=0, pattern=[[1, CH]],
                    channel_multiplier=-1)
                nc.vector.tensor_copy(masks, mtmp)

                def a_unit(t):
                    def emit():
                        x_tm = ft.tile([128, C], f32, tag="xtm", bufs=2,
                                       name=f"x_tm{t}")
                        nc.sync.dma_start(
                            x_tm, x_d.ap()[t * 128:(t + 1) * 128, :])
                        bst = ft.tile([128, 3, 6], f32, tag="bnst", bufs=2,
                                      name="bst")
                        for g in range(3):
                            nc.vector.bn_stats(
                                bst[:, g, :], x_tm[:, g * 256:(g + 1) * 256])
                        mv = ft.tile([128, 2], f32, tag="mv", bufs=2,
                                     name="mv")
                        nc.vector.bn_aggr(mv, bst)
                        lv = ft.tile([128, 1], f32, tag="lv", bufs=2,
                                     name="lv")
                        nc.scalar.activation(lv, mv[:, 1:2], AF.Ln,
                                             bias=eps128)
                        rstd = ft.tile([128, 1], f32, tag="rstd", bufs=2,
                                       name="rstd")
                        nc.scalar.activation(rstd, lv, AF.Exp, scale=-0.5)
                        negmu = ft.tile([128, 1], f32, tag="negmu", bufs=2,
                                        name="negmu")
                        nc.vector.tensor_scalar_mul(negmu, mv[:, 0:1], -1.0)
                        h_tm = ft.tile([128, C], f32, tag="htm", bufs=2,
                                       name="h_tm")
                        nc.vector.tensor_scalar(
                            out=h_tm, in0=x_tm, scalar1=negmu, scalar2=rstd,
                            op0=ALU.add, op1=ALU.mult)
                        c, tj = t // 4, t % 4
                        for f in range(F):
                            fs = slice(f * 128, (f + 1) * 128)
                            ps1 = tr_ps_tile()
                            nc.tensor.transpose(ps1, x_tm[:, fs], ident)
                            nc.scalar.activation(
                                x_fm[c][:, f, tj * 128:(tj + 1) * 128],
                                ps1, AF.Copy)
                            ps2 = tr_ps_tile()
                            nc.tensor.transpose(ps2, h_tm[:, fs], ident)
                            nc.vector.tensor_scalar(
                                out=h_fm[:, f, t * 128:(t + 1) * 128],
                                in0=ps2, scalar1=lnw_c[:, f:f + 1],
                                scalar2=lnb_c[:, f:f + 1],
                                op0=ALU.mult, op1=ALU.add)
                    return emit

                with nc.named_scope("phA"):
                    for t in range(4):
                        a_unit(t)()
                    _interleave([a_unit(t) for t in range(4, 8)],
                                b_units(0))

            # ---------------- attention machinery ----------------
            # NOTE: heads run sequentially; alternating base-0/base-64
            # scores matmuls back-to-back silently corrupts the PE output
            # on this hardware (row-group concurrency issue).
            def _attnv1(f, c, hl, kt, e, sub):
                ktmax = 4 * (c + 1)
                h = 2 * f + hl
                nc.tensor.matmul(
                    st["ys"][(f, c, hl)][:, sub],
                    v1[:, kt, h * 65:(h + 1) * 65], e[:, sub],
                    start=(kt == 0), stop=(kt == ktmax - 1))

            def attn_step1(f, c, kt, hl):
                def emit():
                    off = c * CH - kt * 128
                    lo = max(0, -off)
                    sub = slice(lo, CH)
                    qt = st["qf"][(f, c)]
                    if kt == 0:
                        st["ys"][(f, c, hl)] = ps_y.tile(
                            [65, CH], f32, tag="y", name=f"y{f}_{c}_{hl}")
                    ps = ps_s.tile([128, CH], f32, tag="s", name="s")
                    nc.tensor.matmul(
                        ps[:, sub],
                        kf[hl * 64:(hl + 1) * 64, f,
                           kt * 128:(kt + 1) * 128],
                        qt[hl * 64:(hl + 1) * 64, sub],
                        start=True, stop=True)
                    e = rot.tile([128, CH], bf16, tag="e", bufs=4,
                                 name="expt")
                    nc.scalar.activation(e[:, sub], ps[:, sub], AF.Exp)
                    if off < 128:
                        nc.vector.tensor_mul(
                            e[:, sub], e[:, sub], masks[:, 0:CH - lo])
                    prev = st["pend"].pop((f, c, hl), None)
                    if prev is not None:
                        _attnv1(f, c, hl, *prev)
                    st["pend"][(f, c, hl)] = (kt, e, sub)
                return emit

            def attn_flush1(f, c, hl):
                def emit():
                    prev = st["pend"].pop((f, c, hl))
                    _attnv1(f, c, hl, *prev)
                    ys = st["ys"].pop((f, c, hl))
                    dn = rot.tile([1, CH], f32, tag="dn", bufs=1, name="dn")
                    nc.vector.tensor_copy(dn, ys[64:65, :])
                    dv = rot.tile([1, CH], f32, tag="dv", bufs=1, name="dv")
                    nc.vector.reciprocal_approx_fast(dv, dn)
                    bcb = rot.tile([128, CH], f32, tag="bcb", bufs=1,
                                   name="bcb")
                    nc.gpsimd.partition_broadcast(bcb[0:64, :], dv)
                    nc.vector.tensor_mul(
                        att_o[c][hl * 64:(hl + 1) * 64, f, :],
                        ys[0:64, :], bcb[0:64, :])
                return emit

            def attn_steps(c):
                us = []
                for f in range(F):
                    for hl in range(2):
                        for kt in range(4 * (c + 1)):
                            us.append(attn_step1(f, c, kt, hl))
                        us.append(attn_flush1(f, c, hl))
                return us

            # ---------------- phase C: attn(c0) || B(c1) ----------------
            if "C" in PHASES:
                with nc.named_scope("phC"):
                    _interleave(attn_steps(0), b_units(1))

        # ---------------- phases D/E ----------------
        with tc.tile_pool(name="mw", bufs=1) as mw:
            h2 = {}

            def outproj_unit(c, ct):
                def emit():
                    sl = slice(c * CH, (c + 1) * CH)
                    wt = mw.tile([128, F, 128], bf16, tag="woutt", bufs=2,
                                 name="woutt")
                    for kt in range(F):
                        nc.sync.dma_start(
                            wt[:, kt, :],
                            dd["w_out"].ap()[kt * 128:(kt + 1) * 128,
                                             ct * 128:(ct + 1) * 128])
                    ps = ps_aux.tile([128, CH], f32, tag="aux", name="ps_o")
                    for kt in range(F):
                        nc.tensor.matmul(
                            ps, wt[:, kt, :], att_o[c][:, kt, :],
                            start=(kt == 0), stop=(kt == F - 1))
                    t1 = mw.tile([128, CH], f32, tag="t1", bufs=2,
                                 name="o_t1")
                    nc.vector.tensor_scalar_add(t1, ps, bout_c[:, ct:ct + 1])
                    nc.vector.tensor_add(x2_fm[:, ct, sl], t1,
                                         x_fm[c][:, ct, :])
                return emit

            def ln2_unit(c):
                def emit():
                    sl = slice(c * CH, (c + 1) * CH)
                    h2[c] = mw.tile([128, F, CH], bf16, tag="h2", bufs=1,
                                    name=f"h2_{c}")
                    ps_sum = ps_aux.tile([1, CH], f32, tag="aux",
                                         name="ln_sum")
                    for kt in range(F):
                        nc.tensor.matmul(
                            ps_sum, ones_col, x2_fm[:, kt, sl],
                            start=(kt == 0), stop=(kt == F - 1))
                    ps_sq = ps_aux.tile([1, CH], f32, tag="aux",
                                        name="ln_sq")
                    for kt in range(F):
                        sq = mw.tile([128, CH], f32r, tag="t1", bufs=2,
                                     name="sq")
                        nc.vector.tensor_mul(sq, x2_fm[:, kt, sl],
                                             x2_fm[:, kt, sl])
                        nc.tensor.matmul(ps_sq, ones_col, sq,
                                         start=(kt == 0), stop=(kt == F - 1))
                    r_mean = mw.tile([1, CH], f32, tag="lnA", bufs=1,
                                     name="r_mean")
                    nc.vector.tensor_scalar_mul(r_mean, ps_sum, 1.0 / C)
                    r_m2 = mw.tile([1, CH], f32, tag="t1", bufs=2,
                                   name="r_m2")
                    nc.vector.tensor_scalar_mul(r_m2, ps_sq, 1.0 / C)
                    r_msq = mw.tile([1, CH], f32, tag="t1", bufs=2,
                                    name="r_msq")
                    nc.vector.tensor_mul(r_msq, r_mean, r_mean)
                    nc.vector.tensor_sub(r_m2, r_m2, r_msq)
                    nmu_r = mw.tile([1, CH], f32r, tag="lnr_a", bufs=1,
                                    name="nmu_r")
                    nc.vector.tensor_scalar_mul(nmu_r, r_mean, -1.0)
                    # reuse r_mean for ln(var+eps) (mean no longer needed)
                    nc.scalar.activation(r_mean, r_m2, AF.Ln, bias=eps1)
                    rstd_r = mw.tile([1, CH], f32r, tag="lnr_b", bufs=1,
                                     name="rstd_r")
                    nc.scalar.activation(rstd_r, r_mean, AF.Exp, scale=-0.5)
                    bcN = ps_aux.tile([128, CH], f32, tag="aux", name="bcN")
                    nc.tensor.matmul(bcN, ones1, nmu_r, start=True,
                                     stop=True)
                    bcR = ps_aux.tile([128, CH], f32, tag="aux", name="bcR")
                    nc.tensor.matmul(bcR, ones1, rstd_r, start=True,
                                     stop=True)
                    for f in range(F):
                        t1 = mw.tile([128, CH], f32, tag="t1", bufs=2,
                                     name="ln_t1")
                        nc.vector.tensor_add(t1, x2_fm[:, f, sl], bcN)
                        nc.vector.tensor_mul(t1, t1, bcR)
                        nc.vector.tensor_scalar(
                            out=h2[c][:, f, :], in0=t1,
                            scalar1=lnw_c[:, f:f + 1],
                            scalar2=lnb_c[:, f:f + 1],
                            op0=ALU.mult, op1=ALU.add)
                return emit

            g_sb = {}

            def pass1_unit(c, mt):
                def emit():
                    sl = slice(c * CH, (c + 1) * CH)
                    if mt == 0:
                        g_sb[c] = mw.tile([128, MT, CH], bf16, tag="g",
                                          bufs=1, name=f"g{c}")
                    if mt % 4 == 0:
                        wg = mw.tile([128, F, 512], bf16, tag="wc1",
                                     bufs=2, name="wc1g")
                        for kt in range(F):
                            nc.sync.dma_start(
                                wg[:, kt, :],
                                dd["w_c1"].ap()[kt * 128:(kt + 1) * 128,
                                                mt * 128:mt * 128 + 512])
                        st[("wc1g", c)] = wg
                    wg = st[("wc1g", c)]
                    m0 = (mt % 4) * 128
                    ps = ps_acc.tile([128, CH], f32, tag="acc", name="ps_g")
                    for kt in range(F):
                        nc.tensor.matmul(ps, wg[:, kt, m0:m0 + 128],
                                         h2[c][:, kt, :],
                                         start=(kt == 0), stop=(kt == F - 1))
                    # bias added now so the deferred gelu batch is bias-free
                    nc.vector.tensor_scalar_add(
                        g_sb[c][:, mt, :], ps, bc1_c[:, mt:mt + 1])
                return emit

            def gelu_units(c, n_batch):
                us = []
                per = MT // n_batch
                for b in range(n_batch):
                    def emit(b=b):
                        g = g_sb[c]
                        nc.scalar.activation(
                            g[:, b * per:(b + 1) * per, :],
                            g[:, b * per:(b + 1) * per, :], AF.Gelu)
                    us.append(emit)
                return us

            def pass2_unit(c, ct):
                def emit():
                    sl = slice(c * CH, (c + 1) * CH)
                    wt = mw.tile([128, MT, 128], bf16, tag="wc2", bufs=2,
                                 name="wc2t")
                    for mt in range(MT):
                        nc.sync.dma_start(
                            wt[:, mt, :],
                            dd["w_c2"].ap()[mt * 128:(mt + 1) * 128,
                                            ct * 128:(ct + 1) * 128])
                    pool, tg = ((ps_s, "s"), (ps_y, "y"))[ct % 2]
                    ps = pool.tile([128, CH], f32, tag=tg, name="ps_m")
                    for mt in range(MT):
                        nc.tensor.matmul(ps, wt[:, mt, :],
                                         g_sb[c][:, mt, :],
                                         start=(mt == 0),
                                         stop=(mt == MT - 1))
                    t1 = mw.tile([128, CH], f32, tag="t1", bufs=2,
                                 name="m_t1")
                    nc.vector.tensor_scalar_add(t1, ps, bc2_c[:, ct:ct + 1])
                    out_fm = st.setdefault(("out", c), None)
                    if out_fm is None:
                        out_fm = rp.tile([128, F, CH], f32, tag=f"xfm{c}",
                                         name=f"out_fm{c}")
                        st[("out", c)] = out_fm
                    nc.vector.tensor_add(out_fm[:, ct, :], t1,
                                         x2_fm[:, ct, sl])
                return emit

            def store_unit(c, tj):
                def emit():
                    t = 4 * c + tj
                    out_fm = st[("out", c)]
                    o_tm = mw.tile([128, C], f32, tag="otm", bufs=1,
                                   name="o_tm")
                    for f in range(F):
                        ps = ps_aux.tile([128, 128], f32, tag="aux",
                                         name="tr")
                        nc.tensor.transpose(
                            ps, out_fm[:, f, tj * 128:(tj + 1) * 128],
                            ident)
                        nc.vector.tensor_copy(
                            o_tm[:, f * 128:(f + 1) * 128], ps)
                    nc.sync.dma_start(
                        y_d.ap()[t * 128:(t + 1) * 128, :], o_tm)
                return emit

            # ---------------- phase D: attn(c1) || proj/LN2/MLP-p1(c0) ---
            if "D" in PHASES:
                with nc.named_scope("phD"):
                    fillers = ([outproj_unit(0, ct) for ct in range(F)]
                               + [ln2_unit(0)]
                               + [pass1_unit(0, mt) for mt in range(MT)])
                    _interleave(attn_steps(1), fillers)

            # ---------------- phase E ----------------
            if "E" not in PHASES:
                return
            with nc.named_scope("phE1"):
                # ln2(c1) rows use the exp table set; gelu(c0) switches after
                for ct in range(F):
                    outproj_unit(1, ct)()
                ln2_unit(1)()
                for u in gelu_units(0, 6):
                    u()
            with nc.named_scope("phE2"):
                _interleave([pass2_unit(0, ct) for ct in range(F)], [])
            with nc.named_scope("phE3"):
                # pass1(c1) with gelu sub-batches + store(c0) interleaved
                p1 = [pass1_unit(1, mt) for mt in range(MT)]
                fillers = [store_unit(0, tj) for tj in range(4)]
                _interleave(p1, fillers)
                for u in gelu_units(1, 6):
                    u()
            with nc.named_scope("phE4"):
                _interleave([pass2_unit(1, ct) for ct in range(F)], [])
                for tj in range(4):
                    store_unit(1, tj)()


def _get_nc():
    global _NC_CACHE
    if _NC_CACHE is None:
        _NC_CACHE = _build()
    return _NC_CACHE


def _prep_shared(inputs):
    f = lambda k: np.ascontiguousarray(np.asarray(inputs[k], np.float32))
    bf = lambda a: np.ascontiguousarray(a.astype(ml_dtypes.bfloat16))
    w_qkv = f("w_qkv")
    return {
        "w_kq": np.ascontiguousarray(w_qkv[:, :2 * C]),
        "w_v": np.ascontiguousarray(w_qkv[:, 2 * C:]),
        "b_qkv": f("b_qkv"),
        "w_out": bf(f("w_out")),
        "b_out": f("b_out"),
        "w_c1": bf(f("w_c1")),
        "b_c1": f("b_c1"),
        "w_c2": bf(f("w_c2")),
        "b_c2": f("b_c2"),
        "ln_w": f("ln_w"),
        "ln_b": f("ln_b"),
    }


def run(inputs, trace=False):
    nc = _get_nc()
    xs = np.ascontiguousarray(np.asarray(inputs["x"], dtype=np.float32))
    assert xs.shape == (B, T, C), xs.shape
    shared = _prep_shared(inputs)
    in_maps = [dict(shared, x=xs[c]) for c in range(B)]
    res = bass_utils.run_bass_kernel_spmd(
        nc, in_maps, core_ids=list(range(B)), trace=trace)
    out = np.stack([r["y"] for r in res.results], axis=0)
    return out, res


def kernel(**inputs):
    out, _ = run(inputs, trace=False)
    return out
